# revision 34
# baseline (speedup 1.0000x reference)
"""Quantized-weight batched linear: out[b,n,m] = sum_k deq(qweight)[n,k] * x[b,k,m].

Strategy (fp8 DoubleRow):
  - Host: dequantize weight (fp32, exact oracle formula), subtract per-row mean
    c[n], transpose to (K, N), round residual + activations to fp8 e4m3
    (TRN flavor, ml_dtypes.float8_e4m3). The rank-1 term c[n] * colsum(x)[b,m]
    is added back on the host in fp32 after the device matmul. Inputs are
    pre-laid-out chunk-contiguous so every DMA is a single large transfer.
  - Device (8 cores, data-parallel over batch B=64 -> 8 batches/core):
    PE fp8 matmuls in DoubleRow mode (2 contraction rows/cycle, K chunks of
    256), accumulated in PSUM over 4 chunks. Batches processed in pairs so
    each weight tile serves 4 matmuls. N tiled 8x128, M tiled 2x512.
    Startup loads round-robin over 3 DMA queues; stores (256KB contiguous)
    alternate over 2 queues. Output stored fp16, upcast + corrected on host.
"""

import numpy as np
import ml_dtypes

N = 1024  # output rows (weight rows)
K = 1024  # reduction dim
M = 1024  # columns of x per batch
NGROUP = 16
GS = K // NGROUP
B = 64
NCORES = 8
BPC = B // NCORES  # batches per core

KC2 = K // 256     # DoubleRow contraction chunks (256 each)
NT = N // 128      # output-row tiles
MT = M // 512      # moving free-dim tiles (one PSUM bank each)

_CACHE = {}
LAST_RESULT = None  # BassKernelResults of the most recent run (for profiling)


def _build_nc(bpc=BPC, k=K, n=N, m=M):
    import concourse.mybir as mybir
    import concourse.tile as tile
    from concourse import bacc

    DR = mybir.MatmulPerfMode.DoubleRow

    nc = bacc.Bacc(None, target_bir_lowering=False, debug=False)
    # wt: weight residual, chunk-contiguous: [c, p, (i, n)] = r.T[(2c+i)*128+p, n]
    wt = nc.dram_tensor("wt", [KC2, 128, 2 * n], mybir.dt.float8e4, kind="ExternalInput")
    # xs: per batch partition-major chunk layout: [b, p, (c, i, m)] = x[b, (2c+i)*128+p, m]
    xs = nc.dram_tensor("xs", [bpc, 128, 2 * KC2 * m], mybir.dt.float8e4, kind="ExternalInput")
    out = nc.dram_tensor("out", [bpc, n, m], mybir.dt.float16, kind="ExternalOutput")

    npairs = bpc // 2

    with tile.TileContext(nc) as tc:
        with (
            tc.tile_pool(name="wpool", bufs=KC2) as wpool,
            tc.tile_pool(name="x0pool", bufs=2 * KC2) as x0pool,
            tc.tile_pool(name="xpool", bufs=4) as xpool,
            tc.tile_pool(name="opool", bufs=12) as opool,
            tc.tile_pool(name="scrpool", bufs=1) as scrpool,
            tc.tile_pool(name="psum", bufs=8, space="PSUM") as psum_pool,
        ):
            load_qs = [nc.sync, nc.scalar, nc.gpsimd]
            lq = [0]

            def load_dma(out_ap, in_ap):
                eng = load_qs[lq[0] % len(load_qs)]
                lq[0] += 1
                eng.dma_start(out=out_ap, in_=in_ap)

            # Startup: pair-0 x and weights as per-chunk 256KB transfers,
            # round-robin across 3 queues, in consumption order: group A uses
            # (w_c, x_b0_c) chunks first; x_b1 chunks are only needed by
            # group B ~7us later, so they must not steal early bandwidth.
            wc = []
            x0c = {0: [], 1: []}
            for c in range(KC2):
                t = wpool.tile([128, 2, n], mybir.dt.float8e4, tag="w", name=f"w{c}")
                load_dma(t[:], wt[c])
                wc.append(t)
                xt = x0pool.tile([128, 2, m], mybir.dt.float8e4, tag="x0", name=f"x0_{c}")
                load_dma(xt[:], xs[0, :, c * 2 * m:(c + 1) * 2 * m])
                x0c[0].append(xt)
            for c in range(KC2):
                xt = x0pool.tile([128, 2, m], mybir.dt.float8e4, tag="x0", name=f"x1_{c}")
                load_dma(xt[:], xs[1, :, c * 2 * m:(c + 1) * 2 * m])
                x0c[1].append(xt)

            # Warm-up: a few dummy DoubleRow matmuls on a memset scratch tile.
            # They depend on no DMA, so they execute during the ~5us startup
            # DMA latency window, pre-warming the PE HAM clock gate (3.4us
            # sustained busy -> 2.4 GHz) before the first real matmul.
            scr = scrpool.tile([128, 2, 512], mybir.dt.float8e4, tag="scr", name="scr")
            nc.vector.memset(scr[:], 0)
            ps_w = psum_pool.tile([128, 512], mybir.dt.float32, tag="ps", name="ps_warm")
            for i in range(6):
                nc.tensor.matmul(
                    ps_w[:], scr[:, :, 0:128], scr[:],
                    start=True, stop=True, perf_mode=DR,
                )

            store_qs = [nc.gpsimd, nc.sync]

            def rhs_ap(xcur, bi, c, m0):
                t = xcur[bi]
                if isinstance(t, list):  # pair 0: per-chunk tiles
                    return t[c][:, :, m0 * 512:(m0 + 1) * 512]
                return t[:, 2 * c:2 * c + 2, m0 * 512:(m0 + 1) * 512]

            xcur = x0c
            sq = 0
            for p in range(npairs):
                b0 = 2 * p

                def prefetch_next(p=p, b0=b0):
                    if p + 1 >= npairs:
                        return None
                    xnext = {}
                    for bi in (0, 1):
                        t = xpool.tile([128, 2 * KC2, m], mybir.dt.float8e4,
                                       tag="x", name=f"x{b0 + 2 + bi}")
                        # 256KB chunks on scalar/gpsimd: keeps the sync queue
                        # free for stores (a 1MB transfer occupies the issuing
                        # sequencer ~2.8us and delays store issue behind it)
                        for c in range(KC2):
                            eng = nc.scalar if (bi + c) % 2 == 0 else nc.gpsimd
                            eng.dma_start(
                                out=t[:, 2 * c:2 * c + 2, :],
                                in_=xs[b0 + 2 + bi, :, c * 2 * m:(c + 1) * 2 * m],
                            )
                        xnext[bi] = t
                    return xnext

                # For pair 0, delay the prefetch until startup loads have
                # drained (issued mid-pair); otherwise issue up front.
                xnext = prefetch_next() if p > 0 else None

                # Groups of (n0 tiles, batch slots, m0 tiles), each <= 8 PSUM
                # banks. Pair 0 uses single-batch groups so the first matmul
                # only needs one w chunk + one x chunk (512KB) of startup DMA.
                # The last pair tapers so the final unoverlapped drain is
                # 2 banks.
                if p == 0:
                    groups = [((0, 1, 2, 3), (0,), (0, 1)), ((0, 1, 2, 3), (1,), (0, 1)),
                              ((4, 5, 6, 7), (0,), (0, 1)), ((4, 5, 6, 7), (1,), (0, 1))]
                elif p == npairs - 1:
                    groups = [((0, 1), (0, 1), (0, 1)), ((2, 3), (0, 1), (0, 1)),
                              ((4, 5), (0, 1), (0, 1)), ((6,), (0, 1), (0, 1)),
                              ((7,), (0, 1), (0,)), ((7,), (0,), (1,)),
                              ((7,), (1,), (1,))]
                else:
                    groups = [((0, 1), (0, 1), (0, 1)), ((2, 3), (0, 1), (0, 1)),
                              ((4, 5), (0, 1), (0, 1)), ((6, 7), (0, 1), (0, 1))]
                for gi, (n0s, bis, m0s) in enumerate(groups):
                    if p == 0 and gi == 2:
                        xnext = prefetch_next()
                    ps = {}
                    for n0 in n0s:
                        for bi in bis:
                            for m0 in m0s:
                                ps[n0, bi, m0] = psum_pool.tile(
                                    [128, 512], mybir.dt.float32,
                                    tag="ps", name=f"ps{p}_{n0}_{bi}_{m0}",
                                )
                    for c in range(KC2):
                        for n0 in n0s:
                            lhsT = wc[c][:, :, n0 * 128:(n0 + 1) * 128]
                            for bi in bis:
                                for m0 in m0s:
                                    nc.tensor.matmul(
                                        ps[n0, bi, m0][:],
                                        lhsT,
                                        rhs_ap(xcur, bi, c, m0),
                                        start=(c == 0),
                                        stop=(c == KC2 - 1),
                                        perf_mode=DR,
                                    )
                    for n0 in n0s:
                        for bi in bis:
                            if len(m0s) == 2:
                                ot = opool.tile([128, m], mybir.dt.float16,
                                                tag="o", name=f"o{p}_{n0}_{bi}")
                                nc.vector.tensor_copy(ot[:, 0:512], ps[n0, bi, 0][:])
                                nc.scalar.copy(ot[:, 512:1024], ps[n0, bi, 1][:])
                                store_qs[sq % 2].dma_start(
                                    out=out[b0 + bi, n0 * 128:(n0 + 1) * 128, :],
                                    in_=ot[:],
                                )
                                sq += 1
                            else:
                                # Taper groups: the very last stores go on the
                                # sync queue (fast completion path); gpsimd's
                                # queue drain has ~2us extra latency past the
                                # final landing, so keep it clear at the end.
                                m0 = m0s[0]
                                ot = opool.tile([128, 512], mybir.dt.float16,
                                                tag="os", name=f"os{p}_{n0}_{bi}_{m0}")
                                cp = nc.vector.tensor_copy if bi == 0 else nc.scalar.copy
                                cp(ot[:], ps[n0, bi, m0][:])
                                eng = nc.sync if m0 == 1 else nc.gpsimd
                                eng.dma_start(
                                    out=out[b0 + bi, n0 * 128:(n0 + 1) * 128,
                                            m0 * 512:(m0 + 1) * 512],
                                    in_=ot[:],
                                )
                xcur = xnext
    nc.compile()
    return nc


def _dequant_w(qweight, qrange, qmin):
    # Matches reference: w = q * qrange + qmin per (row, group), fp32.
    q = np.asarray(qweight).astype(np.float32).reshape(N, NGROUP, GS)
    qr = np.asarray(qrange).astype(np.float32).reshape(N, NGROUP, 1)
    qm = np.asarray(qmin).astype(np.float32).reshape(N, NGROUP, 1)
    return (q * qr + qm).reshape(N, K)


def _ensure_axon_hooks():
    """run_bass_kernel_spmd(trace=True) imports antenv.axon_hooks, which some
    images lack; provide a stub (and register the real NTFF hook if the boot
    package is present) so tracing degrades gracefully instead of crashing."""
    try:
        import antenv.axon_hooks  # noqa: F401
        return
    except ImportError:
        pass
    try:
        import sys
        import types

        import antenv

        mod = types.ModuleType("antenv.axon_hooks")
        mod._hook = None
        mod.set_axon_ntff_profile_hook = lambda h: setattr(mod, "_hook", h)
        mod.get_axon_ntff_profile_hook = lambda: mod._hook
        sys.modules["antenv.axon_hooks"] = mod
        antenv.axon_hooks = mod
        try:
            from trn_agent_boot.trn_boot import _ntff_profile_via_ctypes

            mod._hook = _ntff_profile_via_ctypes("/opt/axon/libaxon_pjrt.so")
        except Exception:
            pass
    except Exception:
        pass


def kernel(x, qweight, qrange, qmin):
    global LAST_RESULT
    _ensure_axon_hooks()
    from concourse.bass_utils import run_bass_kernel_spmd

    x = np.asarray(x).astype(np.float32, copy=False)
    w = _dequant_w(qweight, qrange, qmin)
    c = w.mean(axis=1)                       # (N,) per-row mean
    r = w - c[:, None]                       # residual, |r| <= ~0.5
    S = x.sum(axis=1)                        # (B, M) exact column sums

    # Weight: chunk-contiguous [c, p, (i, n)] = r.T[(2c+i)*128+p, n]
    wt8 = (np.ascontiguousarray(r.T).astype(ml_dtypes.float8_e4m3)
           .reshape(KC2, 2, 128, N).transpose(0, 2, 1, 3).reshape(KC2, 128, 2 * N))
    wt8 = np.ascontiguousarray(wt8)
    # x: per batch partition-major [b, p, (c, i, m)] = x[b, (2c+i)*128+p, m]
    x8 = (x.astype(ml_dtypes.float8_e4m3)
          .reshape(B, KC2, 2, 128, M).transpose(0, 3, 1, 2, 4).reshape(B, 128, 2 * KC2 * M))

    if "nc" not in _CACHE:
        _CACHE["nc"] = _build_nc()
    nc = _CACHE["nc"]

    in_maps = [
        {"wt": wt8, "xs": np.ascontiguousarray(x8[ci * BPC:(ci + 1) * BPC])}
        for ci in range(NCORES)
    ]
    LAST_RESULT = run_bass_kernel_spmd(nc, in_maps, core_ids=list(range(NCORES)))

    result = np.empty((B, N, M), np.float32)
    for ci in range(NCORES):
        o16 = LAST_RESULT.results[ci]["out"]  # (BPC, N, M) fp16
        for bi in range(BPC):
            b = ci * BPC + bi
            result[b] = o16[bi].astype(np.float32) + c[:, None] * S[b][None, :]
    return result


# revision 35
# speedup vs baseline: 1.0455x; 1.0455x over previous
"""Quantized-weight batched linear: out[b,n,m] = sum_k deq(qweight)[n,k] * x[b,k,m].

Strategy (fp8 DoubleRow):
  - Host: dequantize weight (fp32, exact oracle formula), subtract per-row mean
    c[n], transpose to (K, N), round residual + activations to fp8 e4m3
    (TRN flavor, ml_dtypes.float8_e4m3). The rank-1 term c[n] * colsum(x)[b,m]
    is added back on the host in fp32 after the device matmul. Inputs are
    pre-laid-out chunk-contiguous so every DMA is a single large transfer.
  - Device (8 cores, data-parallel over batch B=64 -> 8 batches/core):
    PE fp8 matmuls in DoubleRow mode (2 contraction rows/cycle, K chunks of
    256), accumulated in PSUM over 4 chunks. Batches processed in pairs so
    each weight tile serves 4 matmuls. N tiled 8x128, M tiled 2x512.
    Startup loads round-robin over 3 DMA queues; stores (256KB contiguous)
    alternate over 2 queues. Output stored fp16, upcast + corrected on host.
"""

import numpy as np
import ml_dtypes

N = 1024  # output rows (weight rows)
K = 1024  # reduction dim
M = 1024  # columns of x per batch
NGROUP = 16
GS = K // NGROUP
B = 64
NCORES = 8
BPC = B // NCORES  # batches per core

KC2 = K // 256     # DoubleRow contraction chunks (256 each)
NT = N // 128      # output-row tiles
MT = M // 512      # moving free-dim tiles (one PSUM bank each)

_CACHE = {}
LAST_RESULT = None  # BassKernelResults of the most recent run (for profiling)


def _build_nc(bpc=BPC, k=K, n=N, m=M):
    import concourse.mybir as mybir
    import concourse.tile as tile
    from concourse import bacc

    DR = mybir.MatmulPerfMode.DoubleRow

    nc = bacc.Bacc(None, target_bir_lowering=False, debug=False)
    # wt: weight residual, chunk-contiguous: [c, p, (i, n)] = r.T[(2c+i)*128+p, n]
    wt = nc.dram_tensor("wt", [KC2, 128, 2 * n], mybir.dt.float8e4, kind="ExternalInput")
    # xs: per batch partition-major chunk layout: [b, p, (c, i, m)] = x[b, (2c+i)*128+p, m]
    xs = nc.dram_tensor("xs", [bpc, 128, 2 * KC2 * m], mybir.dt.float8e4, kind="ExternalInput")
    out = nc.dram_tensor("out", [bpc, n, m], mybir.dt.float16, kind="ExternalOutput")

    npairs = bpc // 2

    with tile.TileContext(nc) as tc:
        with (
            tc.tile_pool(name="wpool", bufs=KC2) as wpool,
            tc.tile_pool(name="x0pool", bufs=2 * KC2) as x0pool,
            tc.tile_pool(name="xpool", bufs=4) as xpool,
            tc.tile_pool(name="opool", bufs=12) as opool,
            tc.tile_pool(name="scrpool", bufs=1) as scrpool,
            tc.tile_pool(name="psum", bufs=8, space="PSUM") as psum_pool,
        ):
            load_qs = [nc.sync, nc.scalar, nc.gpsimd]
            lq = [0]

            def load_dma(out_ap, in_ap):
                eng = load_qs[lq[0] % len(load_qs)]
                lq[0] += 1
                eng.dma_start(out=out_ap, in_=in_ap)

            # Startup: pair-0 x and weights as per-chunk 256KB transfers,
            # round-robin across 3 queues, in consumption order: group A uses
            # (w_c, x_b0_c) chunks first; x_b1 chunks are only needed by
            # group B ~7us later, so they must not steal early bandwidth.
            wc = []
            x0c = {0: [], 1: []}
            for c in range(KC2):
                t = wpool.tile([128, 2, n], mybir.dt.float8e4, tag="w", name=f"w{c}")
                load_dma(t[:], wt[c])
                wc.append(t)
                xt = x0pool.tile([128, 2, m], mybir.dt.float8e4, tag="x0", name=f"x0_{c}")
                load_dma(xt[:], xs[0, :, c * 2 * m:(c + 1) * 2 * m])
                x0c[0].append(xt)
            for c in range(KC2):
                xt = x0pool.tile([128, 2, m], mybir.dt.float8e4, tag="x0", name=f"x1_{c}")
                load_dma(xt[:], xs[1, :, c * 2 * m:(c + 1) * 2 * m])
                x0c[1].append(xt)

            # Warm-up: a few dummy DoubleRow matmuls on a memset scratch tile.
            # They depend on no DMA, so they execute during the ~5us startup
            # DMA latency window, pre-warming the PE HAM clock gate (3.4us
            # sustained busy -> 2.4 GHz) before the first real matmul.
            scr = scrpool.tile([128, 2, 512], mybir.dt.float8e4, tag="scr", name="scr")
            nc.vector.memset(scr[:], 0)
            ps_w = psum_pool.tile([128, 512], mybir.dt.float32, tag="ps", name="ps_warm")
            for i in range(6):
                nc.tensor.matmul(
                    ps_w[:], scr[:, :, 0:128], scr[:],
                    start=True, stop=True, perf_mode=DR,
                )

            store_qs = [nc.gpsimd, nc.sync]

            def rhs_ap(xcur, bi, c, m0):
                t = xcur[bi]
                if isinstance(t, list):  # pair 0: per-chunk tiles
                    return t[c][:, :, m0 * 512:(m0 + 1) * 512]
                return t[:, 2 * c:2 * c + 2, m0 * 512:(m0 + 1) * 512]

            xcur = x0c
            sq = 0
            for p in range(npairs):
                b0 = 2 * p

                def prefetch_next(p=p, b0=b0):
                    if p + 1 >= npairs:
                        return None
                    xnext = {}
                    for bi in (0, 1):
                        t = xpool.tile([128, 2 * KC2, m], mybir.dt.float8e4,
                                       tag="x", name=f"x{b0 + 2 + bi}")
                        nc.sync.dma_start(out=t[:], in_=xs[b0 + 2 + bi])
                        xnext[bi] = t
                    return xnext

                # For pair 0, delay the prefetch until startup loads have
                # drained (issued mid-pair); otherwise issue up front.
                xnext = prefetch_next() if p > 0 else None

                # Groups of (n0 tiles, batch slots, m0 tiles), each <= 8 PSUM
                # banks. Pair 0 uses single-batch groups so the first matmul
                # only needs one w chunk + one x chunk (512KB) of startup DMA.
                # The last pair tapers so the final unoverlapped drain is
                # 2 banks.
                if p == 0:
                    groups = [((0, 1, 2, 3), (0,), (0, 1)), ((0, 1, 2, 3), (1,), (0, 1)),
                              ((4, 5, 6, 7), (0,), (0, 1)), ((4, 5, 6, 7), (1,), (0, 1))]
                elif p == npairs - 1:
                    groups = [((0, 1), (0, 1), (0, 1)), ((2, 3), (0, 1), (0, 1)),
                              ((4, 5), (0, 1), (0, 1)), ((6,), (0, 1), (0, 1)),
                              ((7,), (0, 1), (0,)), ((7,), (0, 1), (1,))]
                else:
                    groups = [((0, 1), (0, 1), (0, 1)), ((2, 3), (0, 1), (0, 1)),
                              ((4, 5), (0, 1), (0, 1)), ((6, 7), (0, 1), (0, 1))]
                for gi, (n0s, bis, m0s) in enumerate(groups):
                    if p == 0 and gi == 2:
                        xnext = prefetch_next()
                    ps = {}
                    for n0 in n0s:
                        for bi in bis:
                            for m0 in m0s:
                                ps[n0, bi, m0] = psum_pool.tile(
                                    [128, 512], mybir.dt.float32,
                                    tag="ps", name=f"ps{p}_{n0}_{bi}_{m0}",
                                )
                    for c in range(KC2):
                        for n0 in n0s:
                            lhsT = wc[c][:, :, n0 * 128:(n0 + 1) * 128]
                            for bi in bis:
                                for m0 in m0s:
                                    nc.tensor.matmul(
                                        ps[n0, bi, m0][:],
                                        lhsT,
                                        rhs_ap(xcur, bi, c, m0),
                                        start=(c == 0),
                                        stop=(c == KC2 - 1),
                                        perf_mode=DR,
                                    )
                    for n0 in n0s:
                        for bi in bis:
                            if len(m0s) == 2:
                                ot = opool.tile([128, m], mybir.dt.float16,
                                                tag="o", name=f"o{p}_{n0}_{bi}")
                                nc.vector.tensor_copy(ot[:, 0:512], ps[n0, bi, 0][:])
                                nc.scalar.copy(ot[:, 512:1024], ps[n0, bi, 1][:])
                                store_qs[sq % 2].dma_start(
                                    out=out[b0 + bi, n0 * 128:(n0 + 1) * 128, :],
                                    in_=ot[:],
                                )
                                sq += 1
                            else:
                                # Taper groups: the very last stores go on the
                                # sync queue (fast completion path); gpsimd's
                                # queue drain has ~2us extra latency past the
                                # final landing, so keep it clear at the end.
                                m0 = m0s[0]
                                ot = opool.tile([128, 512], mybir.dt.float16,
                                                tag="os", name=f"os{p}_{n0}_{bi}_{m0}")
                                cp = nc.vector.tensor_copy if bi == 0 else nc.scalar.copy
                                cp(ot[:], ps[n0, bi, m0][:])
                                eng = nc.sync if m0 == 1 else nc.gpsimd
                                eng.dma_start(
                                    out=out[b0 + bi, n0 * 128:(n0 + 1) * 128,
                                            m0 * 512:(m0 + 1) * 512],
                                    in_=ot[:],
                                )
                xcur = xnext
    nc.compile()
    return nc


def _dequant_w(qweight, qrange, qmin):
    # Matches reference: w = q * qrange + qmin per (row, group), fp32.
    q = np.asarray(qweight).astype(np.float32).reshape(N, NGROUP, GS)
    qr = np.asarray(qrange).astype(np.float32).reshape(N, NGROUP, 1)
    qm = np.asarray(qmin).astype(np.float32).reshape(N, NGROUP, 1)
    return (q * qr + qm).reshape(N, K)


def _ensure_axon_hooks():
    """run_bass_kernel_spmd(trace=True) imports antenv.axon_hooks, which some
    images lack; provide a stub (and register the real NTFF hook if the boot
    package is present) so tracing degrades gracefully instead of crashing."""
    try:
        import antenv.axon_hooks  # noqa: F401
        return
    except ImportError:
        pass
    try:
        import sys
        import types

        import antenv

        mod = types.ModuleType("antenv.axon_hooks")
        mod._hook = None
        mod.set_axon_ntff_profile_hook = lambda h: setattr(mod, "_hook", h)
        mod.get_axon_ntff_profile_hook = lambda: mod._hook
        sys.modules["antenv.axon_hooks"] = mod
        antenv.axon_hooks = mod
        try:
            from trn_agent_boot.trn_boot import _ntff_profile_via_ctypes

            mod._hook = _ntff_profile_via_ctypes("/opt/axon/libaxon_pjrt.so")
        except Exception:
            pass
    except Exception:
        pass


def kernel(x, qweight, qrange, qmin):
    global LAST_RESULT
    _ensure_axon_hooks()
    from concourse.bass_utils import run_bass_kernel_spmd

    x = np.asarray(x).astype(np.float32, copy=False)
    w = _dequant_w(qweight, qrange, qmin)
    c = w.mean(axis=1)                       # (N,) per-row mean
    r = w - c[:, None]                       # residual, |r| <= ~0.5
    S = x.sum(axis=1)                        # (B, M) exact column sums

    # Weight: chunk-contiguous [c, p, (i, n)] = r.T[(2c+i)*128+p, n]
    wt8 = (np.ascontiguousarray(r.T).astype(ml_dtypes.float8_e4m3)
           .reshape(KC2, 2, 128, N).transpose(0, 2, 1, 3).reshape(KC2, 128, 2 * N))
    wt8 = np.ascontiguousarray(wt8)
    # x: per batch partition-major [b, p, (c, i, m)] = x[b, (2c+i)*128+p, m]
    x8 = (x.astype(ml_dtypes.float8_e4m3)
          .reshape(B, KC2, 2, 128, M).transpose(0, 3, 1, 2, 4).reshape(B, 128, 2 * KC2 * M))

    if "nc" not in _CACHE:
        _CACHE["nc"] = _build_nc()
    nc = _CACHE["nc"]

    in_maps = [
        {"wt": wt8, "xs": np.ascontiguousarray(x8[ci * BPC:(ci + 1) * BPC])}
        for ci in range(NCORES)
    ]
    LAST_RESULT = run_bass_kernel_spmd(nc, in_maps, core_ids=list(range(NCORES)))

    result = np.empty((B, N, M), np.float32)
    for ci in range(NCORES):
        o16 = LAST_RESULT.results[ci]["out"]  # (BPC, N, M) fp16
        for bi in range(BPC):
            b = ci * BPC + bi
            result[b] = o16[bi].astype(np.float32) + c[:, None] * S[b][None, :]
    return result
